# revision 1
# baseline (speedup 1.0000x reference)
import sys
sys.path.insert(0, '/opt/trn_rl_repo')
import numpy as np
import ml_dtypes

import concourse.bass as bass
import concourse.mybir as mybir
from concourse.bass_utils import run_bass_kernel_spmd

# Problem: y[b,s,o] = x[b]@W.T + bias + (x[b]@a[idx[b]].T)@b[idx[b]].T
# B=8 batch elements -> data-parallel, one per NeuronCore.
B, S, D, RANK = 8, 2048, 4096, 16
P = 128
KT = D // P          # 32 contraction tiles
NQ = 4               # s-quarters
SQ = S // NQ         # 512
NJ = 8               # o-blocks of 512
OJ = D // NJ         # 512
NT = SQ // P         # 4 s-tiles per quarter
NGROUP = NQ * NJ * NT  # 128 output groups of [128 s, 512 o]

_BF = mybir.dt.bfloat16
_F32 = mybir.dt.float32


def build_nc():
    nc = bass.Bass()
    xt = nc.declare_dram_parameter("xt", [D, S], _BF, isOutput=False)
    wt = nc.declare_dram_parameter("wt", [D, D], _BF, isOutput=False)
    at = nc.declare_dram_parameter("at", [D, RANK], _BF, isOutput=False)
    bt = nc.declare_dram_parameter("bt", [RANK + 1, D], _BF, isOutput=False)
    ones = nc.declare_dram_parameter("ones", [1, S], _BF, isOutput=False)
    y = nc.declare_dram_parameter("y", [S, D], _F32, isOutput=True)

    xt_t = xt.rearrange("(k p) s -> p k s", p=P)
    wt_t = wt.rearrange("(k p) o -> p k o", p=P)
    at_t = at.rearrange("(k p) r -> p k r", p=P)

    with (
        nc.sbuf_tensor([P, 2, KT, SQ], _BF) as x_sb,
        nc.sbuf_tensor([P, 2, KT, OJ], _BF) as w_sb,
        nc.sbuf_tensor([P, KT, RANK], _BF) as at_sb,
        nc.sbuf_tensor([RANK + 1, D], _BF) as bt_sb,
        nc.sbuf_tensor([RANK + 1, S], _BF) as inter_sb,
        nc.sbuf_tensor([P, 4, OJ], _F32) as out_sb,
        nc.psum_tensor([P, 7, OJ], _F32) as psum_y,
        nc.psum_tensor([P, SQ], _F32) as psum_i,
        nc.semaphore("x_sem") as x_sem,
        nc.semaphore("w_sem") as w_sem,
        nc.semaphore("c_sem") as c_sem,
        nc.semaphore("pe_sem") as pe_sem,
        nc.semaphore("pei_sem") as pei_sem,
        nc.semaphore("dve_sem") as dve_sem,
        nc.semaphore("ev_sem") as ev_sem,
        nc.semaphore("st_sem") as st_sem,
        nc.Block() as block,
    ):
        @block.sync
        def _(sync):
            sync.dma_start(at_sb[:], at_t).then_inc(c_sem, 16)
            sync.dma_start(bt_sb[:], bt[:, :]).then_inc(c_sem, 16)
            sync.dma_start(inter_sb[RANK:RANK + 1, :], ones[:, :]).then_inc(c_sem, 16)
            for q in range(NQ):
                if q >= 2:
                    sync.wait_ge(ev_sem, NJ * NT * (q - 1))
                sync.dma_start(
                    x_sb[:, q % 2], xt_t[:, :, q * SQ:(q + 1) * SQ]
                ).then_inc(x_sem, 16)
                for j in range(NJ):
                    wj = q * NJ + j
                    if wj >= 2:
                        sync.wait_ge(ev_sem, NT * (wj - 1))
                    sync.dma_start(
                        w_sb[:, j % 2], wt_t[:, :, j * OJ:(j + 1) * OJ]
                    ).then_inc(w_sem, 16)

        @block.tensor
        def _(tensor):
            tensor.wait_ge(c_sem, 48)
            g = 0
            for q in range(NQ):
                tensor.wait_ge(x_sem, 16 * (q + 1))
                if q > 0:
                    tensor.wait_ge(dve_sem, q)     # psum_i WAR
                for i in range(KT):
                    mm = nc.tensor.matmul(
                        psum_i[0:RANK, :], at_sb[:, i, :], x_sb[:, q % 2, i, :],
                        start=(i == 0), stop=(i == KT - 1),
                    )
                mm.then_inc(pei_sem, 1)
                for j in range(NJ):
                    wj = q * NJ + j
                    tensor.wait_ge(w_sem, 16 * (wj + 1))
                    for t in range(NT):
                        st = q * NT + t
                        if g >= 7:
                            tensor.wait_ge(ev_sem, g - 6)
                        for i in range(KT):
                            nc.tensor.matmul(
                                psum_y[:, g % 7, :],
                                x_sb[:, q % 2, i, t * P:(t + 1) * P],
                                w_sb[:, j % 2, i, :],
                                start=(i == 0), stop=False,
                            )
                        tensor.wait_ge(dve_sem, q + 1)
                        nc.tensor.matmul(
                            psum_y[:, g % 7, :],
                            inter_sb[:, st * P:(st + 1) * P],
                            bt_sb[:, j * OJ:(j + 1) * OJ],
                            start=False, stop=True,
                        ).then_inc(pe_sem, 1)
                        g += 1

        @block.vector
        def _(vector):
            for q in range(NQ):
                vector.wait_ge(pei_sem, q + 1)
                nc.vector.tensor_copy(
                    inter_sb[0:RANK, q * SQ:(q + 1) * SQ], psum_i[0:RANK, :]
                ).then_inc(dve_sem, 1)

        @block.scalar
        def _(scalar):
            for g in range(NGROUP):
                scalar.wait_ge(pe_sem, g + 1)
                if g >= 4:
                    scalar.wait_ge(st_sem, 16 * (g - 3))
                nc.scalar.copy(out_sb[:, g % 4, :], psum_y[:, g % 7, :]).then_inc(
                    ev_sem, 1
                )

        @block.gpsimd
        def _(gpsimd):
            for g in range(NGROUP):
                q, rem = divmod(g, NJ * NT)
                j, t = divmod(rem, NT)
                st = q * NT + t
                gpsimd.wait_ge(ev_sem, g + 1)
                gpsimd.dma_start(
                    y[st * P:(st + 1) * P, j * OJ:(j + 1) * OJ], out_sb[:, g % 4, :]
                ).then_inc(st_sem, 16)

    return nc


_NC_CACHE = {}


def _get_nc():
    if "nc" not in _NC_CACHE:
        _NC_CACHE["nc"] = build_nc()
    return _NC_CACHE["nc"]


def make_in_maps(x, W, bias, lora_a, lora_b, adapter_indices):
    wt = np.ascontiguousarray(W.astype(np.float32).T).astype(ml_dtypes.bfloat16)
    ones = np.ones((1, S), dtype=ml_dtypes.bfloat16)
    in_maps = []
    for c in range(B):
        idx = int(adapter_indices[c])
        xt = np.ascontiguousarray(x[c].astype(np.float32).T).astype(ml_dtypes.bfloat16)
        at = np.ascontiguousarray(lora_a[idx].astype(np.float32).T).astype(
            ml_dtypes.bfloat16)
        bt = np.concatenate(
            [lora_b[idx].astype(np.float32).T, bias.astype(np.float32)[None, :]],
            axis=0).astype(ml_dtypes.bfloat16)
        in_maps.append({"xt": xt, "wt": wt, "at": at, "bt": bt, "ones": ones})
    return in_maps


def kernel(x, W, bias, lora_a, lora_b, adapter_indices):
    nc = _get_nc()
    in_maps = make_in_maps(x, W, bias, lora_a, lora_b, adapter_indices)
    res = run_bass_kernel_spmd(nc, in_maps, list(range(B)))
    out = np.stack([res.results[c]["y"] for c in range(B)], axis=0)
    return out.astype(np.float32)
